# revision 12
# baseline (speedup 1.0000x reference)
"""Trainium2 Bass kernel for token-level contrastive loss (CLIP-style with
softmax token pooling), distributed over 8 NeuronCores.

v4 design: shard the token axis T (196 -> padded 200 = 8 cores x 25 slices).
The host pre-normalizes tokens (fp32), scales by 16 and ships fp8e4m3 in
[d%128, t, d//128, b] layout, which is exactly the DoubleRow [Ki, Ko=2, *]
interleave for the D=256 contraction.  Per core:

  - dots: one DoubleRow fp8 matmul per (b-tile i, t) -> [128, 512] f32 PSUM,
    2 t's per 2-bank PSUM tile, triple buffered for deep ACT/DVE overlap.
  - e = exp(dots/256) on ScalarE (fp8 out), tmp = (dots/256)*e on DVE via
    one scalar_tensor_tensor per group (fp8 out).  e and tmp interleave in
    one [128, 26, (e|tmp), 512] tile.  Only Exp/Copy activations are used
    -> a single ACT table load.
  - S = sum_t e and V = sum_t tmp accumulate in one [128, (S|V), 512] f32
    PSUM tile per b-tile via DoubleRow "stacked identity" matmuls: each
    instruction folds a PAIR of t slices (contraction over Ko=2) and emits
    S and V together (FD=1024).
  - per-i flush: ACT copy f32->f16 -> DMA straight to an output tensor.
    No device collectives at all: each core returns its partial S/V and the
    host does the 8-way add, pad correction and the bidirectional InfoNCE
    loss in numpy (O(B^2) work).
"""

import sys

sys.path.insert(0, "/opt/trn_rl_repo")

import numpy as np

import concourse.bass as bass
import concourse.mybir as mybir
import concourse.tile as tile
from concourse import bacc
from concourse.bass import ds, ts
from concourse.bass_utils import run_bass_kernel_spmd
from concourse.masks import make_identity

B = 512
T = 196
D = 256
NCORES = 8
TPAD = 200
TLOC = TPAD // NCORES  # 25
NB = B // 128          # 4 b-tiles
NPAD = TPAD - T        # 4 zero pad slices globally (all land on core 7)
TEMP = 0.07
EPS = 1e-8
SCALE = 16.0           # host pre-scale of normalized tokens
INV = 1.0 / (SCALE * SCALE)

NPAIR = (TLOC + 1) // 2  # 13 t-pairs per b-tile (last pair padded with zeros)
NSLOT = 2 * NPAIR        # 26 e/tmp slots

F32 = mybir.dt.float32
F16 = mybir.dt.float16
FP8 = mybir.dt.float8e4

GROUPS = [3] * 8 + [1]   # t-group sizes per b-tile (8*3 + 1 = 25)
# group index -> (first pair, pair count) accumulation bursts
# pair p needs t=2p+1 done; group g ends at t=3g+2
BURSTS = {1: (0, 3), 3: (3, 3), 5: (6, 3), 7: (9, 3), 8: (12, 1)}


def _build_program():
    nc = bacc.Bacc(
        "TRN2",
        target_bir_lowering=False,
        debug=False,
        num_devices=NCORES,
    )
    # host-pretransposed, normalized, fp8: [p=d%128, t, h=d//128, b]
    text_in = nc.dram_tensor("text", [128, TLOC, 2, B], FP8, kind="ExternalInput")
    vis_in = nc.dram_tensor("vis", [128, TLOC, 2, B], FP8, kind="ExternalInput")
    # per-i partial sums: [128, (S|V), B] f16
    outs = [
        nc.dram_tensor(f"sv{i}", [128, 2, B], F16, kind="ExternalOutput")
        for i in range(NB)
    ]

    with tile.TileContext(nc) as tc:
        with (
            tc.tile_pool(name="const", bufs=1) as cpool,
            tc.tile_pool(name="tok", bufs=1) as tokpool,
            tc.tile_pool(name="et", bufs=2) as etpool,
            tc.tile_pool(name="svf", bufs=2) as svfpool,
            tc.tile_pool(name="psd", bufs=2, space="PSUM") as psd,
            tc.tile_pool(name="pssv", bufs=1, space="PSUM") as pssv,
        ):
            # ---- constants: stacked DoubleRow identity [Ki=128, Ko=2, 128] ----
            ident2 = cpool.tile([128, 2, 128], FP8, tag="ident2")
            nc.gpsimd.memset(ident2[:], 0.0)
            make_identity(nc, ident2[:, 0, :], nomemset=True)
            make_identity(nc, ident2[:, 1, :], nomemset=True)

            # ---- persistent SBUF token tiles ----
            texT = tokpool.tile([128, TLOC, 2, B], FP8, tag="texT")
            visT = tokpool.tile([128, TLOC, 2, B], FP8, tag="visT")

            # ---- input loads: alternating text/vis chunks in ascending t,
            # small first chunk so compute starts early ----
            for a, b in ((0, 2), (2, 7), (7, 13), (13, 19), (19, 25)):
                tg = ds(a, b - a)
                nc.sync.dma_start(texT[:, tg, :, :], text_in.ap()[:, tg, :, :])
                nc.sync.dma_start(visT[:, tg, :, :], vis_in.ap()[:, tg, :, :])

            DR = mybir.MatmulPerfMode.DoubleRow

            for i in range(NB):
                # interleaved slots: [p, t-slot, (e|tmp), c]
                etmp = etpool.tile([128, NSLOT, 2, B], FP8, tag="etmp")
                # zero the pad slot so the last DoubleRow pair adds 0
                nc.gpsimd.memset(etmp[:, TLOC, :, :], 0.0)
                sv_ps = pssv.tile([128, 2, B], F32, tag="sv")

                def emit_pairs(p0, np_, etmp=etmp, sv_ps=sv_ps):
                    for p in range(p0, p0 + np_):
                        for h in range(2):  # 0: S += e pair, 1: V += tmp pair
                            nc.tensor.matmul(
                                sv_ps[:, h, :],
                                ident2[:],
                                etmp[:, ds(2 * p, 2), h, :],
                                start=(p == 0), stop=(p == NPAIR - 1),
                                perf_mode=DR, skip_group_check=True,
                            )

                t0 = 0
                for gi, gsz in enumerate(GROUPS):
                    dots = psd.tile([128, 3, B], F32, tag="dots")
                    for j in range(gsz):
                        t = t0 + j
                        nc.tensor.matmul(
                            dots[:, j, :],
                            texT[:, t, :, ts(i, 128)],
                            visT[:, t, :, :],
                            start=True, stop=True,
                            perf_mode=DR, skip_group_check=True,
                        )
                    nc.scalar.activation(
                        etmp[:, ds(t0, gsz), 0, :], dots[:, ds(0, gsz), :],
                        mybir.ActivationFunctionType.Exp, scale=INV,
                    )
                    nc.vector.scalar_tensor_tensor(
                        out=etmp[:, ds(t0, gsz), 1, :],
                        in0=dots[:, ds(0, gsz), :],
                        scalar=INV,
                        in1=etmp[:, ds(t0, gsz), 0, :],
                        op0=mybir.AluOpType.mult,
                        op1=mybir.AluOpType.mult,
                    )
                    t0 += gsz
                    if gi in BURSTS:
                        emit_pairs(*BURSTS[gi])

                # ---- flush S/V: f32 PSUM -> f16 SBUF -> DRAM output ----
                sv_sb = svfpool.tile([128, 2, B], F16, tag="sv16")
                nc.scalar.activation(
                    sv_sb[:], sv_ps[:], mybir.ActivationFunctionType.Copy,
                )
                # out-DMA from the scalar queue: descriptor gen runs right
                # behind the flush ACT, no sync-queue latency on the tail
                nc.scalar.dma_start(outs[i].ap(), sv_sb[:])

    nc.compile()
    return nc


_CACHE = {}


def _get_program():
    if "nc" not in _CACHE:
        _CACHE["nc"] = _build_program()
    return _CACHE["nc"]


def _prep_core_inputs(text: np.ndarray, vis: np.ndarray):
    """Normalize per token (fp32), pad T, scale, cast fp8, transpose to
    [p=d%128, t, h=d//128, b] per core."""
    import ml_dtypes

    fp8 = ml_dtypes.float8_e4m3fn

    def prep(x):
        n = np.sqrt(np.einsum("btd,btd->bt", x, x, dtype=np.float32))
        xn = x * (SCALE / np.maximum(n, EPS))[:, :, None]
        xp = np.zeros((B, TPAD, D), np.float32)
        xp[:, :T] = xn
        return xp.astype(fp8)

    tq = prep(text)
    vq = prep(vis)

    in_maps = []
    for k in range(NCORES):
        sl = slice(k * TLOC, (k + 1) * TLOC)
        core = {}
        for name, arr in (("text", tq[:, sl]), ("vis", vq[:, sl])):
            # [b, t, d] -> [d, t, b] -> [h, p, t, b] -> [p, t, h, b]
            x = arr.transpose(2, 1, 0).reshape(2, 128, TLOC, B)
            core[name] = np.ascontiguousarray(x.transpose(1, 2, 0, 3))
        in_maps.append(core)
    return in_maps


def _finish_host(results):
    """Sum per-core partial S/V and compute the loss."""
    S = np.zeros((B, B), np.float32)
    V = np.zeros((B, B), np.float32)
    for i in range(NB):
        rows = slice(128 * i, 128 * i + 128)
        for k in range(NCORES):
            sv = np.asarray(results[k][f"sv{i}"], np.float32)  # [128, 2, B]
            S[rows] += sv[:, 0, :]
            V[rows] += sv[:, 1, :]
    S -= NPAD  # zero pad tokens contributed exp(0)=1 each to S
    sim = V / S
    logits = (sim / TEMP).astype(np.float64)
    diag = np.arange(B)
    row_lse = np.log(np.sum(np.exp(logits), axis=1))
    col_lse = np.log(np.sum(np.exp(logits), axis=0))
    loss = 0.5 * (np.mean(row_lse - logits[diag, diag])
                  + np.mean(col_lse - logits[diag, diag]))
    return np.float32(loss)


def kernel(text_tokens: np.ndarray, visual_tokens: np.ndarray) -> np.ndarray:
    text = np.ascontiguousarray(np.asarray(text_tokens, dtype=np.float32))
    vis = np.ascontiguousarray(np.asarray(visual_tokens, dtype=np.float32))
    assert text.shape == (B, T, D) and vis.shape == (B, T, D)

    in_maps = _prep_core_inputs(text, vis)
    nc = _get_program()
    res = run_bass_kernel_spmd(nc, in_maps, core_ids=list(range(NCORES)))
    loss = _finish_host(res.results)
    return np.asarray(loss, dtype=np.float32).reshape(())


# revision 13
# speedup vs baseline: 1.2965x; 1.2965x over previous
"""Trainium2 Bass kernel for token-level contrastive loss (CLIP-style with
softmax token pooling), distributed over 8 NeuronCores.

v4 design: shard the token axis T (196 -> padded 200 = 8 cores x 25 slices).
The host pre-normalizes tokens (fp32), scales by 16 and ships fp8e4m3 in
[d%128, t, d//128, b] layout, which is exactly the DoubleRow [Ki, Ko=2, *]
interleave for the D=256 contraction.  Per core:

  - dots: one DoubleRow fp8 matmul per (b-tile i, t) -> [128, 512] f32 PSUM,
    2 t's per 2-bank PSUM tile, triple buffered for deep ACT/DVE overlap.
  - e = exp(dots/256) on ScalarE (fp8 out), tmp = (dots/256)*e on DVE via
    one scalar_tensor_tensor per group (fp8 out).  e and tmp interleave in
    one [128, 26, (e|tmp), 512] tile.  Only Exp/Copy activations are used
    -> a single ACT table load.
  - S = sum_t e and V = sum_t tmp accumulate in one [128, (S|V), 512] f32
    PSUM tile per b-tile via DoubleRow "stacked identity" matmuls: each
    instruction folds a PAIR of t slices (contraction over Ko=2) and emits
    S and V together (FD=1024).
  - per-i flush: ACT copy f32->f16 -> DMA straight to an output tensor.
    No device collectives at all: each core returns its partial S/V and the
    host does the 8-way add, pad correction and the bidirectional InfoNCE
    loss in numpy (O(B^2) work).
"""

import sys

sys.path.insert(0, "/opt/trn_rl_repo")

import numpy as np

import concourse.bass as bass
import concourse.mybir as mybir
import concourse.tile as tile
from concourse import bacc
from concourse.bass import ds, ts
from concourse.bass_utils import run_bass_kernel_spmd
from concourse.masks import make_identity

B = 512
T = 196
D = 256
NCORES = 8
TPAD = 200
TLOC = TPAD // NCORES  # 25
NB = B // 128          # 4 b-tiles
NPAD = TPAD - T        # 4 zero pad slices globally (all land on core 7)
TEMP = 0.07
EPS = 1e-8
SCALE = 16.0           # host pre-scale of normalized tokens
INV = 1.0 / (SCALE * SCALE)

NPAIR = (TLOC + 1) // 2  # 13 t-pairs per b-tile (last pair padded with zeros)
NSLOT = 2 * NPAIR        # 26 e/tmp slots

F32 = mybir.dt.float32
F16 = mybir.dt.float16
FP8 = mybir.dt.float8e4

GROUPS = [2] * 12 + [1]  # t-group sizes per b-tile (12*2 + 1 = 25)
# group index -> (first pair, pair count) accumulation bursts
BURSTS = {2: (0, 3), 5: (3, 3), 8: (6, 3), 11: (9, 3), 12: (12, 1)}


def _build_program():
    nc = bacc.Bacc(
        "TRN2",
        target_bir_lowering=False,
        debug=False,
        num_devices=NCORES,
    )
    # host-pretransposed, normalized, fp8: [p=d%128, t, h=d//128, b]
    text_in = nc.dram_tensor("text", [128, TLOC, 2, B], FP8, kind="ExternalInput")
    vis_in = nc.dram_tensor("vis", [128, TLOC, 2, B], FP8, kind="ExternalInput")
    # per-i partial sums: [128, (S|V), B] f16
    outs = [
        nc.dram_tensor(f"sv{i}", [128, 2, B], F16, kind="ExternalOutput")
        for i in range(NB)
    ]

    with tile.TileContext(nc) as tc:
        with (
            tc.tile_pool(name="const", bufs=1) as cpool,
            tc.tile_pool(name="tok", bufs=1) as tokpool,
            tc.tile_pool(name="et", bufs=2) as etpool,
            tc.tile_pool(name="svf", bufs=2) as svfpool,
            tc.tile_pool(name="psd", bufs=3, space="PSUM") as psd,
            tc.tile_pool(name="pssv", bufs=1, space="PSUM") as pssv,
        ):
            # ---- constants: stacked DoubleRow identity [Ki=128, Ko=2, 128] ----
            ident2 = cpool.tile([128, 2, 128], FP8, tag="ident2")
            nc.gpsimd.memset(ident2[:], 0.0)
            make_identity(nc, ident2[:, 0, :], nomemset=True)
            make_identity(nc, ident2[:, 1, :], nomemset=True)

            # ---- persistent SBUF token tiles ----
            texT = tokpool.tile([128, TLOC, 2, B], FP8, tag="texT")
            visT = tokpool.tile([128, TLOC, 2, B], FP8, tag="visT")

            # ---- input loads: alternating text/vis chunks in ascending t,
            # small first chunk so compute starts early ----
            for a, b in ((0, 2), (2, 7), (7, 13), (13, 19), (19, 25)):
                tg = ds(a, b - a)
                nc.sync.dma_start(texT[:, tg, :, :], text_in.ap()[:, tg, :, :])
                nc.sync.dma_start(visT[:, tg, :, :], vis_in.ap()[:, tg, :, :])

            DR = mybir.MatmulPerfMode.DoubleRow

            for i in range(NB):
                # interleaved slots: [p, t-slot, (e|tmp), c]
                etmp = etpool.tile([128, NSLOT, 2, B], FP8, tag="etmp")
                # zero the pad slot so the last DoubleRow pair adds 0
                nc.gpsimd.memset(etmp[:, TLOC, :, :], 0.0)
                sv_ps = pssv.tile([128, 2, B], F32, tag="sv")

                def emit_pairs(p0, np_, etmp=etmp, sv_ps=sv_ps):
                    for p in range(p0, p0 + np_):
                        for h in range(2):  # 0: S += e pair, 1: V += tmp pair
                            nc.tensor.matmul(
                                sv_ps[:, h, :],
                                ident2[:],
                                etmp[:, ds(2 * p, 2), h, :],
                                start=(p == 0), stop=(p == NPAIR - 1),
                                perf_mode=DR, skip_group_check=True,
                            )

                t0 = 0
                for gi, gsz in enumerate(GROUPS):
                    dots = psd.tile([128, 2, B], F32, tag="dots")
                    for j in range(gsz):
                        t = t0 + j
                        nc.tensor.matmul(
                            dots[:, j, :],
                            texT[:, t, :, ts(i, 128)],
                            visT[:, t, :, :],
                            start=True, stop=True,
                            perf_mode=DR, skip_group_check=True,
                        )
                    nc.scalar.activation(
                        etmp[:, ds(t0, gsz), 0, :], dots[:, ds(0, gsz), :],
                        mybir.ActivationFunctionType.Exp, scale=INV,
                    )
                    nc.vector.scalar_tensor_tensor(
                        out=etmp[:, ds(t0, gsz), 1, :],
                        in0=dots[:, ds(0, gsz), :],
                        scalar=INV,
                        in1=etmp[:, ds(t0, gsz), 0, :],
                        op0=mybir.AluOpType.mult,
                        op1=mybir.AluOpType.mult,
                    )
                    t0 += gsz
                    if gi in BURSTS:
                        emit_pairs(*BURSTS[gi])

                # ---- flush S/V: f32 PSUM -> f16 SBUF -> DRAM output ----
                sv_sb = svfpool.tile([128, 2, B], F16, tag="sv16")
                nc.scalar.activation(
                    sv_sb[:], sv_ps[:], mybir.ActivationFunctionType.Copy,
                )
                # out-DMA from the scalar queue: descriptor gen runs right
                # behind the flush ACT, no sync-queue latency on the tail
                nc.scalar.dma_start(outs[i].ap(), sv_sb[:])

    nc.compile()
    return nc


_CACHE = {}


def _get_program():
    if "nc" not in _CACHE:
        _CACHE["nc"] = _build_program()
    return _CACHE["nc"]


def _prep_core_inputs(text: np.ndarray, vis: np.ndarray):
    """Normalize per token (fp32), pad T, scale, cast fp8, transpose to
    [p=d%128, t, h=d//128, b] per core."""
    import ml_dtypes

    fp8 = ml_dtypes.float8_e4m3fn

    def prep(x):
        n = np.sqrt(np.einsum("btd,btd->bt", x, x, dtype=np.float32))
        xn = x * (SCALE / np.maximum(n, EPS))[:, :, None]
        xp = np.zeros((B, TPAD, D), np.float32)
        xp[:, :T] = xn
        return xp.astype(fp8)

    tq = prep(text)
    vq = prep(vis)

    in_maps = []
    for k in range(NCORES):
        sl = slice(k * TLOC, (k + 1) * TLOC)
        core = {}
        for name, arr in (("text", tq[:, sl]), ("vis", vq[:, sl])):
            # [b, t, d] -> [d, t, b] -> [h, p, t, b] -> [p, t, h, b]
            x = arr.transpose(2, 1, 0).reshape(2, 128, TLOC, B)
            core[name] = np.ascontiguousarray(x.transpose(1, 2, 0, 3))
        in_maps.append(core)
    return in_maps


def _finish_host(results):
    """Sum per-core partial S/V and compute the loss."""
    S = np.zeros((B, B), np.float32)
    V = np.zeros((B, B), np.float32)
    for i in range(NB):
        rows = slice(128 * i, 128 * i + 128)
        for k in range(NCORES):
            sv = np.asarray(results[k][f"sv{i}"], np.float32)  # [128, 2, B]
            S[rows] += sv[:, 0, :]
            V[rows] += sv[:, 1, :]
    S -= NPAD  # zero pad tokens contributed exp(0)=1 each to S
    sim = V / S
    logits = (sim / TEMP).astype(np.float64)
    diag = np.arange(B)
    row_lse = np.log(np.sum(np.exp(logits), axis=1))
    col_lse = np.log(np.sum(np.exp(logits), axis=0))
    loss = 0.5 * (np.mean(row_lse - logits[diag, diag])
                  + np.mean(col_lse - logits[diag, diag]))
    return np.float32(loss)


def kernel(text_tokens: np.ndarray, visual_tokens: np.ndarray) -> np.ndarray:
    text = np.ascontiguousarray(np.asarray(text_tokens, dtype=np.float32))
    vis = np.ascontiguousarray(np.asarray(visual_tokens, dtype=np.float32))
    assert text.shape == (B, T, D) and vis.shape == (B, T, D)

    in_maps = _prep_core_inputs(text, vis)
    nc = _get_program()
    res = run_bass_kernel_spmd(nc, in_maps, core_ids=list(range(NCORES)))
    loss = _finish_host(res.results)
    return np.asarray(loss, dtype=np.float32).reshape(())
